# revision 10
# baseline (speedup 1.0000x reference)
"""Long-context attention for TRN2: exact softmax attention.

Full inputs: query/key/value [2, 2048, 16, 128] fp32; output [2, 2048, 16, 128] fp32.
Sharding: the 2*16 = 32 (batch, head) pairs are split 4-per-core across 8 cores
(mathematically equivalent to the hinted ring+Ulysses decomposition, but with
zero inter-core communication).

Per-core Bass kernel, per (b,h) pair:
  scoresT[k, q] = K Q^T  via matmul(lhsT=KT chunk [d,128], rhs=QT [d,512])
  probsT = exp(scale * scoresT)   (ScalarE, fp16 out)
  out[q, 0:128] + sums[q] = probsT^T @ [V | ones]  (PV matmul, ones-column fused)
  out = out * 1/sums   (DVE reciprocal + tensor_scalar_mul)

Layout prep (Q/K transposed to [d, s], V padded with a ones column, fp16 cast)
is done host-side in numpy.
"""

import numpy as np

import concourse.bass as bass  # noqa: F401
import concourse.tile as tile
from concourse import bacc, mybir
from concourse.bass_utils import run_bass_kernel_spmd

B, S, H, D = 2, 2048, 16, 128
PAIRS = B * H          # 32 (b, h) pairs
N_CORES = 8
HPC = PAIRS // N_CORES  # 4 pairs per core
KC = S // 128           # 16 key chunks of 128
QB = 512                # q block for scores matmuls
NQB = S // QB           # 4
VW = 132                # V chunk padded: 128 V cols + 1 ones col + 3 pad
SCALE = 1.0 / float(np.sqrt(D))

_NC_CACHE = None


def _build():
    nc = bacc.Bacc("TRN2", target_bir_lowering=False, debug=False)

    qT_d = nc.dram_tensor("qT", [HPC, D, S], mybir.dt.float16, kind="ExternalInput")
    kT_d = nc.dram_tensor("kT", [HPC, D, S], mybir.dt.float16, kind="ExternalInput")
    vo_d = nc.dram_tensor("vo", [HPC, KC, 128, VW], mybir.dt.float16, kind="ExternalInput")
    out_d = nc.dram_tensor("out", [HPC, S, D], mybir.dt.float32, kind="ExternalOutput")

    with tile.TileContext(nc) as tc:
        with (
            tc.tile_pool(name="qk", bufs=2) as qk_pool,
            tc.tile_pool(name="vones", bufs=3) as v_pool,
            tc.tile_pool(name="probs", bufs=2) as probs_pool,
            tc.tile_pool(name="outs", bufs=4) as out_pool,
            tc.tile_pool(name="small", bufs=4) as small_pool,
            tc.tile_pool(name="spsum", bufs=4, space="PSUM") as scores_psum,
            tc.tile_pool(name="ppsum", bufs=2, space="PSUM") as pv_psum,
        ):
            qT_s, kT_s, vo_s, probs = {}, {}, {}, {}

            def load_head(h):
                qT_s[h] = qk_pool.tile([D, S], mybir.dt.float16, name=f"qT{h}", tag="qT")
                kT_s[h] = qk_pool.tile([D, S], mybir.dt.float16, name=f"kT{h}", tag="kT")
                vo_s[h] = v_pool.tile(
                    [128, KC, VW], mybir.dt.float16, name=f"vo{h}", tag="vo"
                )
                nc.gpsimd.dma_start(qT_s[h][:], qT_d[h, :, :])
                nc.gpsimd.dma_start(kT_s[h][:], kT_d[h, :, :])
                for c in range(KC):
                    nc.gpsimd.dma_start(vo_s[h][:, c, :], vo_d[h, c, :, :])

            def scores_chunk(h, kc):
                # scoresT [k=128, q] for one key chunk, then exp -> probsT fp16
                for qc in range(NQB):
                    sp = scores_psum.tile(
                        [128, QB], mybir.dt.float32, name="sp", tag="sp"
                    )
                    nc.tensor.matmul(
                        sp[:],
                        kT_s[h][:, kc * 128:(kc + 1) * 128],
                        qT_s[h][:, qc * QB:(qc + 1) * QB],
                        start=True,
                        stop=True,
                    )
                    nc.scalar.activation(
                        probs[h][:, kc, qc * QB:(qc + 1) * QB],
                        sp[:],
                        mybir.ActivationFunctionType.Exp,
                        scale=SCALE,
                    )

            def pv_chunk(h, qi):
                # out[q 128, 0:128] = P^T V ; out[:, 128] = row sums of P^T
                # padded to a full 2KB PSUM bank so the two bufs land in
                # distinct banks (accumulation-group isolation)
                ppfull = pv_psum.tile(
                    [128, 512], mybir.dt.float32, name="pp", tag="pp"
                )
                pp = ppfull[:, 0:129]
                for kc in range(KC):
                    nc.tensor.matmul(
                        pp[:],
                        probs[h][:, kc, qi * 128:(qi + 1) * 128],
                        vo_s[h][:, kc, 0:129],
                        start=(kc == 0),
                        stop=(kc == KC - 1),
                    )
                rec = small_pool.tile([128, 1], mybir.dt.float32, name="rec", tag="rec")
                nc.vector.reciprocal(rec[:], pp[:, 128:129])
                ot = out_pool.tile([128, D], mybir.dt.float32, name="ot", tag="ot")
                nc.vector.tensor_scalar_mul(ot[:], pp[:, 0:128], rec[:])
                nc.gpsimd.dma_start(out_d[h, qi * 128:(qi + 1) * 128, :], ot[:])

            # Software pipeline: scores(h) interleaved with PV(h-1) per chunk;
            # head h+1 inputs prefetched during head h compute.
            for i in range(HPC + 1):
                if i == 0:
                    load_head(0)
                if i + 1 < HPC:
                    load_head(i + 1)
                if i < HPC:
                    probs[i] = probs_pool.tile(
                        [128, KC, S], mybir.dt.float16, name=f"probs{i}", tag="probs"
                    )
                for c in range(KC):
                    if i < HPC:
                        scores_chunk(i, c)
                    if i > 0:
                        pv_chunk(i - 1, c)

    nc.compile()
    return nc


def _get_nc():
    global _NC_CACHE
    if _NC_CACHE is None:
        _NC_CACHE = _build()
    return _NC_CACHE


def _make_in_maps(query, key, value):
    q32 = np.asarray(query, dtype=np.float32)
    k32 = np.asarray(key, dtype=np.float32)
    v32 = np.asarray(value, dtype=np.float32)

    qT = q32.transpose(0, 2, 3, 1).astype(np.float16).reshape(PAIRS, D, S)
    kT = k32.transpose(0, 2, 3, 1).astype(np.float16).reshape(PAIRS, D, S)
    vo = np.zeros((PAIRS, KC, 128, VW), np.float16)
    vo[..., :D] = v32.transpose(0, 2, 1, 3).reshape(PAIRS, KC, 128, D)
    vo[..., D] = 1.0

    return [
        {
            "qT": qT[c * HPC:(c + 1) * HPC],
            "kT": kT[c * HPC:(c + 1) * HPC],
            "vo": vo[c * HPC:(c + 1) * HPC],
        }
        for c in range(N_CORES)
    ]


def _gather(results):
    outs = np.stack([results[c]["out"] for c in range(N_CORES)])  # [8, HPC, S, D]
    out = outs.reshape(B, H, S, D).transpose(0, 2, 1, 3)  # [B, S, H, D]
    return np.ascontiguousarray(out).astype(np.float32)


def run(query, key, value, **spmd_kwargs):
    in_maps = _make_in_maps(query, key, value)
    res = run_bass_kernel_spmd(
        _get_nc(), in_maps, core_ids=list(range(N_CORES)), **spmd_kwargs
    )
    return _gather(res.results), res


def kernel(query, key, value):
    out, _ = run(query, key, value)
    return out


# revision 24
# speedup vs baseline: 1.0379x; 1.0379x over previous
"""Long-context attention for TRN2: exact softmax attention.

Full inputs: query/key/value [2, 2048, 16, 128] fp32; output [2, 2048, 16, 128] fp32.
Sharding: the 2*16 = 32 (batch, head) pairs are split 4-per-core across 8 cores
(mathematically equivalent to the hinted ring+Ulysses decomposition, but with
zero inter-core communication).

Per-core Bass kernel, per (b,h) pair:
  scoresT[k, q] = K Q^T  via matmul(lhsT=KT chunk [d,128], rhs=QT [d,512])
  probsT = exp(scale * scoresT)   (ScalarE, fp16 out)
  out[q, 0:128] + sums[q] = probsT^T @ [V | ones]  (PV matmul, ones-column fused)
  out = out * 1/sums   (DVE reciprocal + tensor_scalar_mul)

Layout prep (Q/K transposed to [d, s], V padded with a ones column, fp16 cast)
is done host-side in numpy.
"""

import numpy as np

import concourse.bass as bass  # noqa: F401
import concourse.tile as tile
from concourse import bacc, mybir
from concourse.bass_utils import run_bass_kernel_spmd

B, S, H, D = 2, 2048, 16, 128
PAIRS = B * H          # 32 (b, h) pairs
N_CORES = 8
HPC = PAIRS // N_CORES  # 4 pairs per core
KC = S // 128           # 16 key chunks of 128
QB = 512                # q block for scores matmuls (max fp32 PSUM moving width)
UQ = 1024               # q width of one pipeline unit (half a head)
NU = HPC * (S // UQ)    # 8 units
EW = 1536               # exp width: one 3-bank PSUM super-slot
NEXP = -(-KC * UQ // EW)  # 11 exps per unit (10x1536 + 1x1024)
VW = 132                # V chunk padded: 128 V cols + 1 ones col + 3 pad
SCALE = 1.0 / float(np.sqrt(D))

_NC_CACHE = None


def _build():
    nc = bacc.Bacc("TRN2", target_bir_lowering=False, debug=False)

    qT_d = nc.dram_tensor("qT", [HPC, D, S], mybir.dt.float16, kind="ExternalInput")
    kT_d = nc.dram_tensor("kT", [HPC, D, S], mybir.dt.float16, kind="ExternalInput")
    vo_d = nc.dram_tensor("vo", [HPC, KC, 128, VW], mybir.dt.float16, kind="ExternalInput")
    out_d = nc.dram_tensor("out", [HPC, S, D], mybir.dt.float32, kind="ExternalOutput")

    with tile.TileContext(nc) as tc:
        with (
            tc.tile_pool(name="qk", bufs=2) as qk_pool,
            tc.tile_pool(name="vones", bufs=3) as v_pool,
            tc.tile_pool(name="probs", bufs=2) as probs_pool,
            tc.tile_pool(name="outs", bufs=4) as out_pool,
            tc.tile_pool(name="small", bufs=4) as small_pool,
            tc.tile_pool(name="spsum", bufs=2, space="PSUM") as scores_psum,
            tc.tile_pool(name="ppsum", bufs=2, space="PSUM") as pv_psum,
        ):
            qT_s, kT_s, vo_s, probs = {}, {}, {}, {}

            def load_head(h, first=False):
                qT_s[h] = qk_pool.tile([D, S], mybir.dt.float16, name=f"qT{h}", tag="qT")
                kT_s[h] = qk_pool.tile([D, S], mybir.dt.float16, name=f"kT{h}", tag="kT")
                vo_s[h] = v_pool.tile(
                    [128, KC, VW], mybir.dt.float16, name=f"vo{h}", tag="vo"
                )
                if first:
                    # land exactly what the first exp pair needs, first
                    nc.gpsimd.dma_start(kT_s[h][:, 0:256], kT_d[h, :, 0:256])
                    nc.gpsimd.dma_start(qT_s[h][:, 0:UQ], qT_d[h, :, 0:UQ])
                    nc.gpsimd.dma_start(kT_s[h][:, 256:S], kT_d[h, :, 256:S])
                    nc.gpsimd.dma_start(qT_s[h][:, UQ:S], qT_d[h, :, UQ:S])
                else:
                    nc.gpsimd.dma_start(qT_s[h][:], qT_d[h, :, :])
                    nc.gpsimd.dma_start(kT_s[h][:], kT_d[h, :, :])
                for c in range(KC):
                    nc.gpsimd.dma_start(vo_s[h][:, c, :], vo_d[h, c, :, :])

            def scores_block(u, j):
                # fill a 3-bank super-slot with EW contiguous (kc, q) score
                # elems, then one wide exp over it
                h, half = divmod(u, 2)
                base = j * EW
                w = min(EW, KC * UQ - base)
                sp = scores_psum.tile([128, EW], mybir.dt.float32, name="sp", tag="sp")
                for m in range(w // QB):
                    elem = base + m * QB
                    kc, qq = divmod(elem, UQ)
                    nc.tensor.matmul(
                        sp[:, m * QB:(m + 1) * QB],
                        kT_s[h][:, kc * 128:(kc + 1) * 128],
                        qT_s[h][:, half * UQ + qq:half * UQ + qq + QB],
                        start=True,
                        stop=True,
                    )
                nc.scalar.activation(
                    probs[u][:, base:base + w],
                    sp[:, 0:w],
                    mybir.ActivationFunctionType.Exp,
                    scale=SCALE,
                )

            def pv_chunk(u, qi):
                # out[q 128, 0:128] = P^T V ; out[:, 128] = row sums of P^T
                h, half = divmod(u, 2)
                qt = half * (UQ // 128) + qi  # q tile index within the head
                # padded to a full 2KB PSUM bank so the two bufs land in
                # distinct banks (accumulation-group isolation)
                ppfull = pv_psum.tile(
                    [128, 512], mybir.dt.float32, name="pp", tag="pp"
                )
                pp = ppfull[:, 0:129]
                for kc in range(KC):
                    q0 = kc * UQ + qi * 128
                    nc.tensor.matmul(
                        pp[:],
                        probs[u][:, q0:q0 + 128],
                        vo_s[h][:, kc, 0:129],
                        start=(kc == 0),
                        stop=(kc == KC - 1),
                    )
                rec = small_pool.tile([128, 1], mybir.dt.float32, name="rec", tag="rec")
                nc.vector.reciprocal(rec[:], pp[:, 128:129])
                ot = out_pool.tile([128, D], mybir.dt.float32, name="ot", tag="ot")
                nc.vector.tensor_scalar_mul(ot[:], pp[:, 0:128], rec[:])
                nc.gpsimd.dma_start(out_d[h, qt * 128:(qt + 1) * 128, :], ot[:])

            # Software pipeline over 8 half-head units: scores(u) interleaved
            # with PV(u-1); head h+1 inputs prefetched during head h's first
            # unit.
            for u in range(NU + 1):
                h, half = divmod(u, 2)
                if u == 0:
                    load_head(0, first=True)
                if u < NU and half == 0 and h + 1 < HPC:
                    load_head(h + 1)
                if u < NU:
                    probs[u] = probs_pool.tile(
                        [128, KC * UQ], mybir.dt.float16, name=f"probs{u}", tag="probs"
                    )
                    for j in range(NEXP):
                        scores_block(u, j)
                        if u > 0 and 2 <= j <= 9:
                            pv_chunk(u - 1, j - 2)
                else:
                    for qi in range(UQ // 128):
                        pv_chunk(u - 1, qi)

    nc.compile()
    return nc


def _get_nc():
    global _NC_CACHE
    if _NC_CACHE is None:
        _NC_CACHE = _build()
    return _NC_CACHE


def _make_in_maps(query, key, value):
    q32 = np.asarray(query, dtype=np.float32)
    k32 = np.asarray(key, dtype=np.float32)
    v32 = np.asarray(value, dtype=np.float32)

    qT = q32.transpose(0, 2, 3, 1).astype(np.float16).reshape(PAIRS, D, S)
    kT = k32.transpose(0, 2, 3, 1).astype(np.float16).reshape(PAIRS, D, S)
    vo = np.zeros((PAIRS, KC, 128, VW), np.float16)
    vo[..., :D] = v32.transpose(0, 2, 1, 3).reshape(PAIRS, KC, 128, D)
    vo[..., D] = 1.0

    return [
        {
            "qT": qT[c * HPC:(c + 1) * HPC],
            "kT": kT[c * HPC:(c + 1) * HPC],
            "vo": vo[c * HPC:(c + 1) * HPC],
        }
        for c in range(N_CORES)
    ]


def _gather(results):
    outs = np.stack([results[c]["out"] for c in range(N_CORES)])  # [8, HPC, S, D]
    out = outs.reshape(B, H, S, D).transpose(0, 2, 1, 3)  # [B, S, H, D]
    return np.ascontiguousarray(out).astype(np.float32)


def run(query, key, value, **spmd_kwargs):
    in_maps = _make_in_maps(query, key, value)
    res = run_bass_kernel_spmd(
        _get_nc(), in_maps, core_ids=list(range(N_CORES)), **spmd_kwargs
    )
    return _gather(res.results), res


def kernel(query, key, value):
    out, _ = run(query, key, value)
    return out
